# revision 6
# baseline (speedup 1.0000x reference)
"""SkeletalPool TRN2 kernel, v6: loads on SP+ACT only (2-way), Pool ring
dedicated to stores + DRAM->DRAM root copies.

Rationale: pure-load probes measured 2 concurrent load streams FASTER
than 3 (342 vs 301 GB/s aggregate) - a third stream adds interference.
So the two HWDGE rings carry only loads, and the Pool/SWDGE ring carries
all bf16 stores (write channel rides concurrently) plus the 4 cast
DRAM->DRAM root copies. Everything else matches v4: 1-pair units, DVE
add + exact in-place halve into bf16 (rel err ~4e-3 vs the 2e-2 gate).

Semaphore rules (see v4): SWDGE-updated sems are SWDGE-exclusive with
exact cumulative targets, one DMA in flight per sem; tin slots = 8 so
slot s belongs to load ring s % 2; all store sems belong to Pool.
"""

import sys

if "/opt/trn_rl_repo" not in sys.path:
    sys.path.insert(0, "/opt/trn_rl_repo")

import numpy as np

import concourse.bass as bass
import concourse.mybir as mybir
from concourse.bass_utils import run_bass_kernel_spmd

N_CORES = 8
B_FULL = 32
B_SHARD = B_FULL // N_CORES  # 4
J_IN = 31
J_OUT = 16
C = 64
T = 4096
P = 128
TT = (C * T) // P  # 2048
N_PAIRS = 15
N_UNITS = B_SHARD * N_PAIRS  # 60 pair units per rep; roots go DRAM->DRAM
NB_IN = 8  # tin slots (even: slot sems load-ring-exclusive)
NB_SUM = 8  # tsum slots (store sems all Pool-owned)

f32 = mybir.dt.float32
bf16 = mybir.dt.bfloat16

_CACHE = {}


def _build_nc(reps: int = 1, out_dt=bf16) -> bass.Bass:
    nc = bass.Bass("TRN2", debug=False, num_devices=N_CORES)
    x = nc.dram_tensor("x", (B_SHARD, J_IN, C, T), f32, kind="ExternalInput")
    out = nc.dram_tensor("out", (B_SHARD, J_OUT, C, T), out_dt, kind="ExternalOutput")
    xp = x.ap().rearrange("b j c (u t) -> b (c u) j t", u=2)  # [4,128,31,2048]
    op = out.ap().rearrange("b j c (u t) -> b (c u) j t", u=2)  # [4,128,16,2048]

    tin = nc.alloc_sbuf_tensor("tin", [P, NB_IN * 2 * TT], f32)
    tsum = nc.alloc_sbuf_tensor("tsum", [P, NB_SUM * TT], out_dt)
    s_load = [nc.alloc_semaphore(f"s_load{i}") for i in range(NB_IN)]
    s_store = [nc.alloc_semaphore(f"s_store{i}") for i in range(NB_SUM)]
    s_root = [nc.alloc_semaphore(f"s_root{i}") for i in range(2)]
    s_add = nc.alloc_semaphore("s_add")
    s_mul = nc.alloc_semaphore("s_mul")

    TOT = reps * N_UNITS
    N_ROOTS = reps * B_SHARD

    def task(g):
        b, k1 = divmod(g % N_UNITS, N_PAIRS)
        return b, k1 + 1  # output joint k; inputs (2k-1, 2k)

    def tin_v(g):
        s = (g % NB_IN) * 2 * TT
        return tin.ap()[:, s : s + 2 * TT].rearrange("p (j t) -> p j t", j=2)

    def tsum_v(g):
        s = (g % NB_SUM) * TT
        return tsum.ap()[:, s : s + TT]

    def load_prog(eng, r):
        for g in range(TOT):
            if g % 2 != r:
                continue
            b, k = task(g)
            if g >= NB_IN:
                # tin slot free once unit g-NB_IN's add has read it
                eng.wait_ge(s_add, g - NB_IN + 1)
            j0 = 2 * k - 1
            eng.dma_start(out=tin_v(g), in_=xp[b, :, j0 : j0 + 2, :]).then_inc(
                s_load[g % NB_IN], 16
            )

    def pool_prog(eng):
        for g in range(TOT):
            if g % N_PAIRS == 0:
                # root joint passes through exactly: DRAM->DRAM cast copy
                # (SWDGE), off the saturated HBM->SBUF load channel.
                ri = g // N_PAIRS
                b = ri % B_SHARD
                if ri >= 2:
                    eng.wait_ge(s_root[ri % 2], 16 * (ri // 2))
                eng.dma_start(out=op[b, :, 0, :], in_=xp[b, :, 0, :]).then_inc(
                    s_root[ri % 2], 16
                )
            b, k = task(g)
            eng.wait_ge(s_mul, g + 1)
            eng.dma_start(out=op[b, :, k, :], in_=tsum_v(g)).then_inc(
                s_store[g % NB_SUM], 16
            )

    with nc.Block() as block:

        @block.sync
        def _(sync):
            load_prog(sync, 0)
            # gate kernel end on all stores and roots; counts are exact
            # because slot reuse serializes same-sem DMAs.
            sync.wait_ge(s_mul, TOT)
            for s in range(NB_SUM):
                sync.wait_ge(s_store[s], 16 * len(range(s, TOT, NB_SUM)))
            for i in range(2):
                sync.wait_ge(s_root[i], 16 * len(range(i, N_ROOTS, 2)))

        @block.scalar
        def _(scalar):
            load_prog(scalar, 1)

        @block.gpsimd
        def _(gpsimd):
            pool_prog(gpsimd)

        @block.vector
        def _(vector):
            for g in range(TOT):
                vector.wait_ge(s_load[g % NB_IN], 16 * (g // NB_IN + 1))
                if g >= NB_SUM:
                    # tsum slot free once unit g-NB_SUM's store completed
                    vector.wait_ge(s_store[g % NB_SUM], 16 * (g // NB_SUM))
                tv = tin_v(g)
                sv = tsum_v(g)
                # DVE pipelines instructions (queue depth 8): explicit sem
                # edge for the RAW add -> mul on the same tile
                vector.tensor_add(out=sv, in0=tv[:, 0, :], in1=tv[:, 1, :]).then_inc(
                    s_add, 1
                )
                vector.wait_ge(s_add, g + 1)
                # in-place halve: exact (power of two), same-AP in/out
                vector.tensor_scalar_mul(sv, sv, 0.5).then_inc(s_mul, 1)

    return nc


def get_nc() -> bass.Bass:
    if "nc" not in _CACHE:
        _CACHE["nc"] = _build_nc(1)
    return _CACHE["nc"]


def kernel(x: np.ndarray, **run_kwargs):
    x = np.ascontiguousarray(np.asarray(x, dtype=np.float32))
    assert x.shape == (B_FULL, J_IN, C, T), x.shape

    nc = get_nc()
    in_maps = [
        {"x": np.ascontiguousarray(x[i * B_SHARD : (i + 1) * B_SHARD])}
        for i in range(N_CORES)
    ]
    res = run_bass_kernel_spmd(nc, in_maps, core_ids=list(range(N_CORES)), **run_kwargs)
    out = np.concatenate(
        [np.asarray(res.results[i]["out"]) for i in range(N_CORES)], axis=0
    ).astype(np.float32)
    _CACHE["last_results"] = res
    return out
